# revision 17
# baseline (speedup 1.0000x reference)
"""Trainium2 Bass kernel for a BasicTransformerBlock (B=2, S=2048, H=768, FF=3072, NH=12).

Sharding: core c handles batch b=c//4, sequence quarter q=c%4 (512 tokens).
Each core computes LN1 + K/V/Q projections for its OWN 512 tokens only; the
K/V slices (fp8, ~0.8MB) are exchanged within each 4-core batch group by a
DRAM-DRAM AllGather CollectiveCompute (first-dim concat, absolute block
order), then read back into SBUF.  Tile models the collective's completion,
so the read-back DMAs and attention are ordinary local dependencies.

Attention activations (k/q/v/attn-weights) are fp8e4m3 (same matmul rate as
bf16 on TRN2, half the SBUF/exchange footprint; softmax averaging washes out
the quantization noise).  The attention inner loop is software-pipelined one
step (scores for chunk j issue before ctx for chunk j-1) so the in-order
tensor queue never blocks on the exp semaphore; the ones-column in V
accumulates the softmax denominator (ctx row 64) for free.
Softmax normalize: vector reciprocal of the denominator row (psum, direct),
gpsimd partition-broadcast, vector multiply into bf16 ctxT tiles.

LN affine params and all biases are folded host-side:
  Wq_eff = diag(ln1_w) Wq, bq_eff = ln1_b@Wq + bq  (same k)
  v carries no bias on device;  bo_eff = (ln1_b@Wv + bv)@Wo + bo
  W1_eff = diag(ln2_w) W1, b1_eff = ln2_b@W1 + b1
All weights are pre-transposed host-side to [partition, ...] layouts so every
weight DMA is one contiguous descriptor per partition.
"""

import numpy as np
import ml_dtypes

import concourse.bass as bass
import concourse.tile as tile
from concourse import bacc, mybir
from concourse.bass import ts, ds
from concourse.alu_op_type import AluOpType
from concourse.bass_utils import run_bass_kernel_spmd

F32 = mybir.dt.float32
BF16 = mybir.dt.bfloat16
F8 = mybir.dt.float8e4
AF = mybir.ActivationFunctionType

H = 768
FF = 3072
NH = 12
DH = 64
B = 2
S = 2048
P = 128
NCORES = 8
TQ = 512          # own tokens per core
NTT = S // TQ     # 4 token tiles per batch
FC = H // P       # 6 feature chunks
FFC = FF // P     # 24 hidden chunks
TKC = S // P      # 16 key token chunks
JP = TKC // 2     # 8 key chunk pairs
HPAIRS = NH // 2  # 6 head pairs
EPS = 1e-6


def _act_recip(nc, out, in_):
    """ACT-LUT reciprocal (bass blocks the wrapper for accuracy reasons;
    for softmax denominators the LUT precision is acceptable)."""
    eng = nc.scalar
    ins = [eng.lower_ap(in_)]
    for arg in (0.0, 1.0, 0.0):  # bias, scale, alpha
        ins.append(mybir.ImmediateValue(dtype=mybir.dt.float32, value=arg))
    return eng.add_instruction(mybir.InstActivation(
        name=nc.get_next_instruction_name(),
        func=AF.Reciprocal,
        ins=ins, outs=[eng.lower_ap(out)]))


def _ln_tail(nc, T, ps_sum, ps_sq, small_pool, ab_pool, eps_tile):
    """From accumulated sum (partition 0) / sqsum (partition 32) rows ->
    broadcast alpha/beta [P,T] tiles."""
    mu = small_pool.tile([1, T], F32, tag="lnsmall")
    nc.vector.tensor_scalar_mul(mu[:], ps_sum, 1.0 / H)
    msq = small_pool.tile([1, T], F32, tag="lnsmall")
    nc.vector.tensor_scalar_mul(msq[:], ps_sq, 1.0 / H)
    var = small_pool.tile([1, T], F32, tag="lnsmall")
    nc.vector.tensor_mul(var[:], mu[:], mu[:])
    nc.vector.tensor_sub(var[:], msq[:], var[:])
    sd = small_pool.tile([1, T], F32, tag="lnsmall")
    nc.scalar.activation(sd[:], var[:], AF.Sqrt, bias=eps_tile[:])
    rsig_bf = small_pool.tile([1, T], BF16, tag="lnsmallbf")
    _act_recip(nc, rsig_bf[:], sd[:])
    beta_bf = small_pool.tile([1, T], BF16, tag="lnsmallbf")
    nc.vector.scalar_tensor_tensor(beta_bf[:], mu[:], -1.0, rsig_bf[:],
                                   AluOpType.mult, AluOpType.mult)
    ab = ab_pool.tile([P, T], BF16, tag="ab")
    nc.gpsimd.partition_broadcast(ab[:], rsig_bf[0:1, :])
    bb = ab_pool.tile([P, T], BF16, tag="bb")
    nc.gpsimd.partition_broadcast(bb[:], beta_bf[0:1, :])
    return ab, bb


def build():
    nc = bacc.Bacc("TRN2", target_bir_lowering=False, debug=False,
                   num_devices=NCORES)

    # pre-transposed weight layouts (one contiguous descriptor per partition)
    latbf_d = nc.dram_tensor("latTbf", [P, FC, TQ], BF16, kind="ExternalInput")
    wq_d = nc.dram_tensor("wq", [P, FC, H], BF16, kind="ExternalInput")
    wk_d = nc.dram_tensor("wk", [P, FC, H], BF16, kind="ExternalInput")
    wv_d = nc.dram_tensor("wv", [P, FC, H], BF16, kind="ExternalInput")
    wo_d = nc.dram_tensor("wo", [P, FC, H], BF16, kind="ExternalInput")
    w1_d = nc.dram_tensor("w1", [P, FFC, FC, P], BF16, kind="ExternalInput")
    w2_d = nc.dram_tensor("w2", [P, FFC, H], BF16, kind="ExternalInput")
    bq_d = nc.dram_tensor("bq", [P, FC], F32, kind="ExternalInput")
    bk_d = nc.dram_tensor("bk", [P, FC], F32, kind="ExternalInput")
    bo_d = nc.dram_tensor("bo", [P, FC], F32, kind="ExternalInput")
    b1_d = nc.dram_tensor("b1", [P, FFC], F32, kind="ExternalInput")
    b2_d = nc.dram_tensor("b2", [P, FC], F32, kind="ExternalInput")
    out_d = nc.dram_tensor("outT", [P, FC, TQ], BF16, kind="ExternalOutput")
    # K/V exchange staging: own K (3072B/row) + own V (3120B/row) packed,
    # AllGather concatenates senders along the first dim.
    kv_stage = nc.dram_tensor("kvstage", [P, 6192], F8, kind="Internal")
    kv_gath = nc.dram_tensor("kvgath", [NTT * P, 6192], F8, kind="Internal")

    with tile.TileContext(nc) as tc:
        with (
            tc.tile_pool(name="consts", bufs=1) as consts,
            tc.tile_pool(name="persist", bufs=1) as persist,
        ):
            # constants
            ones_col_bf = consts.tile([P, 1], BF16)
            nc.vector.memset(ones_col_bf[:], 1.0)
            eps_tile = consts.tile([1, 1], F32)
            nc.vector.memset(eps_tile[:], EPS)
            zero_col = consts.tile([P, 1], F32)
            nc.vector.memset(zero_col[:], 0.0)
            bq_sb = consts.tile([P, FC], F32)
            nc.scalar.dma_start(bq_sb[:], bq_d.ap())
            bk_sb = consts.tile([P, FC], F32)
            nc.scalar.dma_start(bk_sb[:], bk_d.ap())
            bo_sb = consts.tile([P, FC], F32)
            nc.scalar.dma_start(bo_sb[:], bo_d.ap())
            b1_sb = consts.tile([P, FFC], F32)
            nc.scalar.dma_start(b1_sb[:], b1_d.ap())
            b2_sb = consts.tile([P, FC], F32)
            nc.scalar.dma_start(b2_sb[:], b2_d.ap())

            # persistent activations (feature-major, fp8)
            # kf8 block-major: [P, token-block, FC, TQ]; local block i holds
            # absolute block (own_q ^ i) -- block 0 is computed locally, the
            # other three arrive from the batch-group peers via remote DMA.
            kf8 = persist.tile([P, NTT, FC, TQ], F8)
            qf8 = persist.tile([P, FC, TQ], F8)
            # v_sb[p, j, h, f] = V[token j*128+p, head h, dim f]; col 64 = 1.0
            # (written entirely by the gather read-back; the ones column is
            # memset in vown and travels with the exchange)
            v_sb = persist.tile([P, TKC, NH, DH + 1], F8)
            ctxT = []
            for hp in range(HPAIRS):
                ctxT_h = persist.tile([P, TQ], BF16, tag=f"ctxT{hp}")
                ctxT.append(ctxT_h)
            lat0bf = persist.tile([P, FC, TQ], BF16)   # own tokens (resid)
            lat2T = persist.tile([P, FC, TQ], F32)
            wo_sb = persist.tile([P, FC, H], BF16)
            nc.scalar.dma_start(wo_sb[:], wo_d.ap())

            # ---------------- Phase 1: LN1 + K/V/Q projections ----------------
            with (
                tc.tile_pool(name="wproj", bufs=1) as wproj,
                tc.tile_pool(name="sqp", bufs=2) as sqp,
                tc.tile_pool(name="nxp", bufs=2) as nxp,
                tc.tile_pool(name="abp", bufs=2) as abp,
                tc.tile_pool(name="smallp", bufs=12) as smallp,
                tc.tile_pool(name="lntmpp", bufs=2) as lntmpp,
                tc.tile_pool(name="ps_stats", bufs=2, space="PSUM") as ps_stats,
                tc.tile_pool(name="ps_kq", bufs=2, space="PSUM") as ps_kq,
                tc.tile_pool(name="ps_v", bufs=2, space="PSUM") as ps_v,
            ):
                # own-token tile first on the sync ring (stats for tile 0
                # gate everything); weights on the scalar ring.
                nc.sync.dma_start(lat0bf[:], latbf_d.ap())
                kown = wproj.tile([P, FC, TQ], F8)
                vown = wproj.tile([P, TQ // P, NH, DH + 1], F8)
                nc.vector.memset(vown[:, :, :, DH:DH + 1], 1.0)
                wk_sb = wproj.tile([P, FC, H], BF16)
                nc.scalar.dma_start(wk_sb[:], wk_d.ap())
                wv_sb = wproj.tile([P, FC, H], BF16)
                nc.scalar.dma_start(wv_sb[:], wv_d.ap())
                wq_sb = wproj.tile([P, FC, H], BF16)
                nc.scalar.dma_start(wq_sb[:], wq_d.ap())

                # LN1 + projections for OWN tokens only (block 0); K/V for
                # the other three blocks arrive from batch-group peers.
                sq_t = sqp.tile([P, FC, TQ], BF16, tag="sq")
                nc.vector.tensor_mul(sq_t[:], lat0bf[:], lat0bf[:])
                ps_stat = ps_stats.tile([33, TQ], F32, tag="stats")
                for c in range(FC):
                    nc.tensor.matmul(ps_stat[0:1, :], ones_col_bf[:],
                                     lat0bf[:, c, :],
                                     start=(c == 0), stop=(c == FC - 1))
                for c in range(FC):
                    nc.tensor.matmul(ps_stat[32:33, :], ones_col_bf[:],
                                     sq_t[:, c, :],
                                     start=(c == 0), stop=(c == FC - 1))
                ab, bb = _ln_tail(nc, TQ, ps_stat[0:1, :], ps_stat[32:33, :],
                                  smallp, abp, eps_tile)
                nx_t = nxp.tile([P, FC, TQ], BF16, tag="nx")
                for c in range(FC):
                    t = lntmpp.tile([P, TQ], BF16, tag="lntmp")
                    nc.vector.tensor_mul(t[:], lat0bf[:, c, :], ab[:])
                    nc.vector.tensor_add(nx_t[:, c, :], t[:], bb[:])

                # K projection (feature-major fp8 out, own block)
                for mc in range(FC):
                    ps = ps_kq.tile([P, TQ], F32, tag="kq")
                    for kc in range(FC):
                        nc.tensor.matmul(ps[:], wk_sb[:, kc, ts(mc, P)],
                                         nx_t[:, kc, :],
                                         start=(kc == 0), stop=(kc == FC - 1))
                    nc.scalar.activation(kown[:, mc, :],
                                         ps[:], AF.Identity,
                                         bias=bk_sb[:, mc:mc + 1])
                # V projection (token-major out, ones col preset, own chunks)
                for tcl in range(TQ // P):
                    for half in range(2):
                        ps = ps_v.tile([P, 384], F32, tag="v")
                        for kc in range(FC):
                            nc.tensor.matmul(ps[:], nx_t[:, kc, ts(tcl, P)],
                                             wv_sb[:, kc, ds(half * 384, 384)],
                                             start=(kc == 0), stop=(kc == FC - 1))
                        nc.scalar.copy(
                            vown[:, tcl, ds(half * 6, 6), 0:DH],
                            ps[:].rearrange("p (h d) -> p h d", d=DH))

                # K/V all-gather within the 4-core batch group via a
                # DRAM-DRAM CollectiveCompute (tile models its completion,
                # so downstream read-back DMAs are ordinary dependencies).
                nc.sync.dma_start(
                    kv_stage.ap()[:, 0:3072],
                    kown[:].rearrange("p c t -> p (c t)"))
                nc.sync.dma_start(
                    kv_stage.ap()[:, 3072:6192],
                    vown[:].rearrange("p c h f -> p (c h f)"))
                nc.gpsimd.collective_compute(
                    "AllGather", mybir.AluOpType.bypass,
                    replica_groups=[[0, 1, 2, 3], [4, 5, 6, 7]],
                    ins=[kv_stage.ap()], outs=[kv_gath.ap()])

                # Q projection (own tokens)
                for mc in range(FC):
                    ps = ps_kq.tile([P, TQ], F32, tag="kq")
                    for kc in range(FC):
                        nc.tensor.matmul(ps[:], wq_sb[:, kc, ts(mc, P)],
                                         nx_t[:, kc, :],
                                         start=(kc == 0), stop=(kc == FC - 1))
                    nc.scalar.activation(qf8[:, mc, :],
                                         ps[:], AF.Identity,
                                         bias=bq_sb[:, mc:mc + 1])

                # read back all four gathered blocks (own included)
                for i in range(NTT):
                    eng = nc.sync if i % 2 == 0 else nc.scalar
                    eng.dma_start(
                        kf8[:, i],
                        kv_gath.ap()[ts(i, P), 0:3072].rearrange(
                            "p (c t) -> p c t", c=FC))
                    eng.dma_start(
                        v_sb[:, ds(4 * i, 4)],
                        kv_gath.ap()[ts(i, P), 3072:6192].rearrange(
                            "p (c h f) -> p c h f", c=TQ // P, h=NH))

            # ------------- Phase 2+3: attention, Wo+LN2, FFN -------------
            # One merged scope: PSUM is exactly 8 banks:
            #   sc tag (2 tiles x [128,2,512]f32)        -> 4 banks
            #   ctx0..ctx3 (1 tile x <=2KB/partition each) -> 4 banks
            # Wo/LN2-stats/FFN draw from the same tags after attention drains.
            with (
                tc.tile_pool(name="a2p", bufs=3) as a2p,
                tc.tile_pool(name="rcp", bufs=4) as rcp,
                tc.tile_pool(name="rbp", bufs=2) as rbp,
                tc.tile_pool(name="ffw", bufs=1) as ffw,
                tc.tile_pool(name="nx2p", bufs=1) as nx2p,
                tc.tile_pool(name="sq2p", bufs=1) as sq2p,
                tc.tile_pool(name="ab2p", bufs=1) as ab2p,
                tc.tile_pool(name="small2p", bufs=6) as small2p,
                tc.tile_pool(name="lntmp2p", bufs=2) as lntmp2p,
                tc.tile_pool(name="hp_pool", bufs=4) as hp_pool,
                tc.tile_pool(name="outp", bufs=1) as outp,
                tc.tile_pool(name="ps_sc", bufs=2, space="PSUM") as ps_sc,
                tc.tile_pool(name="ps_ctx", bufs=1, space="PSUM") as ps_ctx,
            ):
                # FFN weights resident; DMA'd now (sync ring idle during attn)
                w1_all = ffw.tile([P, FFC, FC, P], BF16)
                nc.sync.dma_start(w1_all[:], w1_d.ap())
                w2_all = ffw.tile([P, FFC, H], BF16)
                nc.sync.dma_start(w2_all[:], w2_d.ap())

                # ---- attention (one-step software pipeline: scores for
                # chunk j issue before ctx for chunk j-1, so the in-order
                # tensor queue never blocks on the exp semaphore) ----
                for hp in range(HPAIRS):
                    hA, hB = 2 * hp, 2 * hp + 1
                    ctxA = ps_ctx.tile([DH + 1, TQ], F32, tag=f"ctxA{hp % 2}",
                                       name=f"ctxA{hp}")
                    ctxB = ps_ctx.tile([DH + 1, TQ], F32, tag=f"ctxB{hp % 2}",
                                       name=f"ctxB{hp}")
                    prev_a2 = None
                    for j in range(TKC):
                        sc = ps_sc.tile([P, 2, TQ], F32, tag="sc")
                        nc.tensor.matmul(sc[:, 0, :],
                                         kf8[0:DH, j // 4, hp, ts(j % 4, P)],
                                         qf8[0:DH, hp, :],
                                         start=True, stop=True)
                        nc.tensor.matmul(sc[:, 1, :],
                                         kf8[DH:P, j // 4, hp, ts(j % 4, P)],
                                         qf8[DH:P, hp, :],
                                         start=True, stop=True)
                        if prev_a2 is not None:
                            nc.tensor.matmul(ctxA[:], v_sb[:, j - 1, hA, :],
                                             prev_a2[:, 0, :], start=(j == 1),
                                             stop=False)
                            nc.tensor.matmul(ctxB[:], v_sb[:, j - 1, hB, :],
                                             prev_a2[:, 1, :], start=(j == 1),
                                             stop=False)
                        a2 = a2p.tile([P, 2, TQ], F8, tag="a2")
                        nc.scalar.activation(a2[:], sc[:], AF.Exp,
                                             scale=0.125, bias=zero_col[:])
                        prev_a2 = a2
                    nc.tensor.matmul(ctxA[:], v_sb[:, TKC - 1, hA, :],
                                     prev_a2[:, 0, :], start=False, stop=True)
                    nc.tensor.matmul(ctxB[:], v_sb[:, TKC - 1, hB, :],
                                     prev_a2[:, 1, :], start=False, stop=True)
                    # normalize: recip of denom row (64), broadcast, multiply
                    for half, ctx_ps in ((0, ctxA), (1, ctxB)):
                        rc = rcp.tile([1, TQ], F32, tag="rc")
                        if hp < HPAIRS - 1:
                            nc.vector.reciprocal(rc[:], ctx_ps[DH:DH + 1, :])
                        else:
                            _act_recip(nc, rc[:], ctx_ps[DH:DH + 1, :])
                        rb = rbp.tile([DH, TQ], F32, tag="rb")
                        nc.gpsimd.partition_broadcast(rb[:], rc[0:1, :])
                        nc.vector.tensor_mul(ctxT[hp][ds(half * DH, DH), :],
                                             ctx_ps[0:DH, :], rb[:])

                # ---- Wo projection + residual + LN2 stats ----
                sq2 = sq2p.tile([P, FC, TQ], BF16, tag="sq2")
                latbf2 = sq2p.tile([P, FC, TQ], BF16, tag="latbf2")
                ps_sum2 = ps_ctx.tile([1, TQ], F32, tag="ctxA0", name="sum2")
                ps_sq2 = ps_ctx.tile([33, TQ], F32, tag="ctxB0", name="sqs2")
                for mc in range(FC):
                    ps = ps_sc.tile([P, TQ], F32, tag="sc", name=f"wops{mc}")
                    for kc in range(FC):
                        nc.tensor.matmul(ps[:], wo_sb[:, kc, ts(mc, P)],
                                         ctxT[kc][:],
                                         start=(kc == 0), stop=(kc == FC - 1))
                    nc.vector.affine_then_add(lat2T[:, mc, :], ps[:],
                                              lat0bf[:, mc, :], 1.0,
                                              bo_sb[:, mc:mc + 1])
                    nc.scalar.copy(latbf2[:, mc, :], lat2T[:, mc, :])
                    nc.vector.tensor_mul(sq2[:, mc, :], lat2T[:, mc, :],
                                         lat2T[:, mc, :])
                    nc.tensor.matmul(ps_sum2[0:1, :], ones_col_bf[:],
                                     latbf2[:, mc, :],
                                     start=(mc == 0), stop=(mc == FC - 1))
                    nc.tensor.matmul(ps_sq2[32:33, :], ones_col_bf[:],
                                     sq2[:, mc, :],
                                     start=(mc == 0), stop=(mc == FC - 1))
                ab2, bb2 = _ln_tail(nc, TQ, ps_sum2[0:1, :],
                                    ps_sq2[32:33, :], small2p, ab2p,
                                    eps_tile)
                nx2T = []
                for c in range(FC):
                    nx2T_c = nx2p.tile([P, TQ], BF16, tag=f"nx2T{c}")
                    t2 = lntmp2p.tile([P, TQ], BF16, tag="lntmp2")
                    nc.vector.tensor_mul(t2[:], latbf2[:, c, :], ab2[:])
                    nc.vector.tensor_add(nx2T_c[:], t2[:], bb2[:])
                    nx2T.append(nx2T_c)

                # ---- FFN ----
                # ps_out: mc pairs (0,1),(2,3) on the two sc-tag tiles,
                # mc 4 -> ctx0, mc 5 -> ctx1; psh alternates ctx2/ctx3.
                ps_o01 = ps_sc.tile([P, 2, TQ], F32, tag="sc", name="pso01")
                ps_o23 = ps_sc.tile([P, 2, TQ], F32, tag="sc", name="pso23")
                ps_o4 = ps_ctx.tile([P, TQ], F32, tag="ctxA0", name="pso4")
                ps_o5 = ps_ctx.tile([P, TQ], F32, tag="ctxB0", name="pso5")
                out_ps = [ps_o01[:, 0, :], ps_o01[:, 1, :],
                          ps_o23[:, 0, :], ps_o23[:, 1, :],
                          ps_o4[:], ps_o5[:]]
                for mh in range(FFC):
                    psh = ps_ctx.tile([P, TQ], F32,
                                      tag=f"ctx{'A' if mh % 2 else 'B'}1",
                                      name=f"h{mh}")
                    for kc in range(FC):
                        nc.tensor.matmul(psh[:], w1_all[:, mh, kc, :],
                                         nx2T[kc][:],
                                         start=(kc == 0), stop=(kc == FC - 1))
                    h_t = hp_pool.tile([P, TQ], BF16, tag="h_sb")
                    nc.scalar.activation(h_t[:], psh[:], AF.Gelu,
                                         bias=b1_sb[:, mh:mh + 1])
                    for mc in range(FC):
                        nc.tensor.matmul(out_ps[mc], w2_all[:, mh, ts(mc, P)],
                                         h_t[:],
                                         start=(mh == 0), stop=(mh == FFC - 1))
                out_engines = [nc.sync, nc.scalar, nc.sync]
                for pc in range(3):
                    outT = outp.tile([P, 2, TQ], BF16, tag=f"out{pc}")
                    for i in range(2):
                        mc = 2 * pc + i
                        nc.vector.affine_then_add(outT[:, i, :], out_ps[mc],
                                                  lat2T[:, mc, :], 1.0,
                                                  b2_sb[:, mc:mc + 1])
                    out_engines[pc].dma_start(out_d.ap()[:, ds(2 * pc, 2), :],
                                              outT[:])

    nc.compile()
    return nc


_NC_CACHE = {}


def _get_nc():
    if "nc" not in _NC_CACHE:
        _NC_CACHE["nc"] = build()
    return _NC_CACHE["nc"]


def _prep_inputs(latent, ln1_w, ln1_b, Wq, bq, Wk, bk, Wv, bv, Wo, bo,
                 ln2_w, ln2_b, W1, b1, W2, b2):
    f32 = np.float32
    bf16 = ml_dtypes.bfloat16
    lat = np.asarray(latent, f32)
    ln1_w = np.asarray(ln1_w, f32); ln1_b = np.asarray(ln1_b, f32)
    ln2_w = np.asarray(ln2_w, f32); ln2_b = np.asarray(ln2_b, f32)
    Wq = np.asarray(Wq, f32); Wk = np.asarray(Wk, f32); Wv = np.asarray(Wv, f32)
    Wo = np.asarray(Wo, f32); W1 = np.asarray(W1, f32); W2 = np.asarray(W2, f32)
    bq = np.asarray(bq, f32); bk = np.asarray(bk, f32); bv = np.asarray(bv, f32)
    bo = np.asarray(bo, f32); b1 = np.asarray(b1, f32); b2 = np.asarray(b2, f32)

    wq_eff = ln1_w[:, None] * Wq
    wk_eff = ln1_w[:, None] * Wk
    wv_eff = ln1_w[:, None] * Wv
    bq_eff = ln1_b @ Wq + bq
    bk_eff = ln1_b @ Wk + bk
    bv_eff = ln1_b @ Wv + bv
    bo_eff = bv_eff @ Wo + bo
    w1_eff = ln2_w[:, None] * W1
    b1_eff = ln2_b @ W1 + b1

    def wT(w):  # [H_in, M] -> [P, FC, M] with w[p, c, m] = w[c*128+p, m]
        return np.ascontiguousarray(
            w.reshape(FC, P, -1).transpose(1, 0, 2)).astype(bf16)

    # w1_all[p, f, c, m] = w1_eff[c*128+p, f*128+m]
    w1_t = np.ascontiguousarray(
        w1_eff.reshape(FC, P, FFC, P).transpose(1, 2, 0, 3)).astype(bf16)
    # w2_all[p, f, m] = W2[f*128+p, m]
    w2_t = np.ascontiguousarray(
        W2.reshape(FFC, P, H).transpose(1, 0, 2)).astype(bf16)

    def chunked(b):  # [H or FF] -> [P, nchunks]
        return np.ascontiguousarray(b.reshape(-1, P).T)

    common = {
        "wq": wT(wq_eff), "wk": wT(wk_eff), "wv": wT(wv_eff), "wo": wT(Wo),
        "w1": w1_t, "w2": w2_t,
        "bq": chunked(bq_eff), "bk": chunked(bk_eff), "bo": chunked(bo_eff),
        "b1": chunked(b1_eff), "b2": chunked(b2),
    }
    in_maps = []
    for c in range(NCORES):
        b = c // (NCORES // B)
        q = c % (NCORES // B)
        # own token block only; K/V for the other quarters arrive on-device
        # via the batch-group AllGather (absolute block order).
        latbf = np.ascontiguousarray(
            lat[b].T.reshape(FC, P, NTT, TQ)[:, :, q].transpose(1, 0, 2)
        ).astype(bf16)
        m = dict(common)
        m["latTbf"] = latbf
        in_maps.append(m)
    return in_maps


def kernel(**inputs):
    nc = _get_nc()
    in_maps = _prep_inputs(**inputs)
    res = run_bass_kernel_spmd(nc, in_maps, core_ids=list(range(NCORES)))
    out = np.empty((B, S, H), np.float32)
    for c in range(NCORES):
        b = c // (NCORES // B)
        q = c % (NCORES // B)
        # outT [P, FC, TQ] bf16 -> [TQ, H] f32 with feature f = c*128+p
        o = np.asarray(res.results[c]["outT"], np.float32)
        out[b, q * TQ:(q + 1) * TQ, :] = o.transpose(2, 1, 0).reshape(TQ, H)
    return out
